# revision 26
# baseline (speedup 1.0000x reference)
"""Compact bilinear pooling (count-sketch + FFT) Trainium2 kernel.

Math: for each image, y = irfft( sum_over_pixels( rfft(x_px @ S1) * rfft(x_px @ S2) ) ),
then signed-sqrt and L2 normalization.  Since rfft(x @ S) == x @ rfft(S), the
per-pixel FFTs become plain matmuls against W = rfft(S, axis=1) (precomputed on
host once per call); the inverse FFT is linear so it is applied AFTER spatial
sum pooling via a factored (Cooley-Tukey, 128x64) real IDFT.

Pipeline per frequency tile (128 bins x 392 pixel-cols):
  PE:   16 bf16 matmuls -> 4 PSUM tiles (f1r f1i f2r f2i)
  ACT:  drains f1r/f1i -> SBUF bf16
  DVE:  drains f2r/f2i -> SBUF bf16, then fused product+pool
        (scalar_tensor_tensor with accum) for the rr/ri terms
  Pool: fused product+pool for the ii/ir terms (SBUF-only operands)
Pooled spectra land in per-term staging columns; one subtract/add pair at the
end forms P_r/P_i.  The IDFT second stage scatters straight into a [128,256]
PSUM accumulator laid out as (img*32 + n_hi, n_lo), so the signed-sqrt +
L2-norm epilogue runs at full 128-partition width.

Sharding: data-parallel over 8 NeuronCores, 4 images each; W / DFT bases are
replicated.  Everything except the rfft(S) weight prep runs on device.
"""

import os
import numpy as np

import concourse.bass as bass
import concourse.bacc as bacc
import concourse.mybir as mybir
import concourse.tile as tile
from concourse.bass_utils import run_bass_kernel_spmd

D = 8192          # projection dim
CH = 512          # input channels
HW = 196          # pixels per image (14x14)
B = 32            # batch
NCORES = 8
BPD = B // NCORES     # images per device (4)
RWS = BPD * HW        # spatial rows per device (784)
NT = 33               # 32 main freq tiles of 128 + 1 nyquist-extension tile
NCOL = NT * 4         # pooled staging columns (t, img)
F32 = mybir.dt.float32

_DR_ON = bool(int(os.environ.get("CBP_DR", "0")))
_DT_NAME = os.environ.get("CBP_MM_DTYPE", "float8e4" if _DR_ON else "bfloat16")
DT_A = getattr(mybir.dt, _DT_NAME)     # phase A projection dtype
DT_MM = mybir.dt.bfloat16              # phase B / drain dtype
_NPA = mybir.dt.np(DT_A)
_NPDT = mybir.dt.np(DT_MM)
_POOL_ON = bool(int(os.environ.get("CBP_POOL", "1")))

AX = mybir.AxisListType
ALU = mybir.AluOpType
ACT = mybir.ActivationFunctionType


def _build():
    nc = bacc.Bacc(None, target_bir_lowering=False)

    xd = nc.dram_tensor("xdev", [128, 4, RWS], DT_A, kind="ExternalInput")
    wallt = nc.dram_tensor("wallt", [NT, 128, 16, 128], DT_A, kind="ExternalInput")
    cbt = nc.dram_tensor("cbt", [128, D], DT_MM, kind="ExternalInput")
    sbt = nc.dram_tensor("sbt", [128, D], DT_MM, kind="ExternalInput")
    cwf = nc.dram_tensor("cwf", [128, 512], F32, kind="ExternalInput")
    swf = nc.dram_tensor("swf", [128, 512], F32, kind="ExternalInput")
    selx = nc.dram_tensor("selx", [128, 160], DT_MM, kind="ExternalInput")
    nyqs = nc.dram_tensor("nyqs", [4, 128], DT_MM, kind="ExternalInput")
    alt2 = nc.dram_tensor("alt2", [4, 256], F32, kind="ExternalInput")
    bones = nc.dram_tensor("bones", [128, 4], DT_MM, kind="ExternalInput")
    ones1 = nc.dram_tensor("ones1", [1, 1], F32, kind="ExternalInput")
    yd = nc.dram_tensor("ydev", [BPD, D], F32, kind="ExternalOutput")

    with tile.TileContext(nc) as tc:
        with tc.tile_pool(name="singles", bufs=1) as singles:
            # x load split across the ACT/gpsimd DGE queues; the sync queue
            # carries only the W-tile stream so w0 lands immediately
            x_sb = singles.tile([128, 4, RWS], DT_A)
            nc.scalar.dma_start(out=x_sb[:, 0, :], in_=xd[:, 0, :])
            nc.scalar.dma_start(out=x_sb[:, 1, :], in_=xd[:, 1, :])
            nc.scalar.dma_start(out=x_sb[:, 2, :], in_=xd[:, 2, :])
            nc.gpsimd.dma_start(out=x_sb[:, 3, :], in_=xd[:, 3, :])
            # pin the sqrt-containing activation table set now so the
            # epilogue Sqrt doesn't trigger a table swap on the critical tail
            actwarm = singles.tile([1, 1], F32)
            nc.vector.memset(actwarm, 1.0)
            nc.scalar.activation(actwarm, actwarm, ACT.Sqrt)

            # back-half constants via the gpsimd (SWDGE) queue so they don't
            # head-block the sync-ring W-tile stream
            cb_sb = singles.tile([128, D], DT_MM)
            nc.gpsimd.dma_start(out=cb_sb, in_=cbt[:, :])
            sb_sb = singles.tile([128, D], DT_MM)
            nc.gpsimd.dma_start(out=sb_sb, in_=sbt[:, :])
            cwf_sb = singles.tile([128, 512], F32)
            nc.gpsimd.dma_start(out=cwf_sb, in_=cwf[:, :])
            swf_sb = singles.tile([128, 512], F32)
            nc.gpsimd.dma_start(out=swf_sb, in_=swf[:, :])
            selx_sb = singles.tile([128, 160], DT_MM)
            nc.gpsimd.dma_start(out=selx_sb, in_=selx[:, :])
            nyqs_sb = singles.tile([4, 128], DT_MM)
            nc.gpsimd.dma_start(out=nyqs_sb, in_=nyqs[:, :])
            alt2_sb = singles.tile([4, 256], F32)
            nc.gpsimd.dma_start(out=alt2_sb, in_=alt2[:, :])
            bones_sb = singles.tile([128, 4], DT_MM)
            nc.gpsimd.dma_start(out=bones_sb, in_=bones[:, :])
            ones1_sb = singles.tile([1, 1], F32)
            nc.gpsimd.dma_start(out=ones1_sb, in_=ones1[:, :])

            # pooled per-term spectra, col = 4*t + img
            sd_rr = singles.tile([128, NCOL], F32)
            sd_ri = singles.tile([128, NCOL], F32)
            sp_ii = singles.tile([128, NCOL], F32)
            sp_ir = singles.tile([128, NCOL], F32)
            # on DVE: the first product stt ops depend on these tiles, and the
            # gpsimd queue is busy streaming constants at this point
            nc.vector.memset(sd_ri[:, 128:132], 0.0)
            nc.vector.memset(sp_ii[:, 128:132], 0.0)
            nc.vector.memset(sp_ir[:, 128:132], 0.0)
            # epilogue bias constants, hoisted off the critical tail
            e1 = singles.tile([128, 1], F32)
            nc.gpsimd.memset(e1, 1e-8)
            e2 = singles.tile([4, 1], F32)
            nc.gpsimd.memset(e2, float(D * 1e-8))

            # ---------------- phase A: projections + pooled spectral products
            with tc.tile_pool(name="wp", bufs=2) as wp, \
                 tc.tile_pool(name="fp", bufs=2, space="PSUM") as fp, \
                 tc.tile_pool(name="cp", bufs=2) as cp, \
                 tc.tile_pool(name="scr", bufs=2) as scr:
                for t in range(NT):
                    w_sb = wp.tile([128, 16, 128], DT_A, tag="w")
                    nc.sync.dma_start(out=w_sb, in_=wallt[t, :, :, :])
                    mlist = (0, 2) if t == 32 else (0, 1, 2, 3)
                    for h in range(2):
                        rsl = slice(h * 392, (h + 1) * 392)
                        fts = {}
                        for m in mlist:
                            ft = fp.tile([128, 392], F32, tag=f"f{m}", name=f"ft{m}")
                            fts[m] = ft
                            if _DR_ON:
                                # DoubleRow: contract 256 channels per matmul
                                # (both cc planes of the pair at once)
                                for g in range(2):
                                    nc.tensor.matmul(
                                        ft,
                                        lhsT=w_sb[:, m * 4 + 2 * g:m * 4 + 2 * g + 2, :],
                                        rhs=x_sb[:, 2 * g:2 * g + 2, rsl],
                                        start=(g == 0),
                                        stop=(g == 1),
                                        perf_mode=mybir.MatmulPerfMode.DoubleRow,
                                    )
                            else:
                                for cc in range(4):
                                    nc.tensor.matmul(
                                        ft,
                                        lhsT=w_sb[:, m * 4 + cc, :],
                                        rhs=x_sb[:, cc, rsl],
                                        start=(cc == 0),
                                        stop=(cc == 3),
                                    )
                        # ACT drains f1 to SBUF (halves DVE PSUM-port traffic
                        # and lets the f1 PSUM banks recycle early)
                        c1r = cp.tile([128, 392], DT_MM, tag="c1r")
                        nc.scalar.copy(c1r, fts[0])
                        if t != 32:
                            c1i = cp.tile([128, 392], DT_MM, tag="c1i")
                            nc.scalar.copy(c1i, fts[1])
                        # fused product+pool on DVE, f2 read straight from
                        # PSUM; col = 4t + 2h + li
                        # order rr, ir first: both read f2r, so its PSUM bank
                        # frees two stt earlier for the next tile's matmuls
                        terms = [(c1r, fts[2], sd_rr, "s1")]
                        if t != 32:
                            terms += [(c1i, fts[2], sp_ir, "s3"),
                                      (c1r, fts[3], sd_ri, "s2"),
                                      (c1i, fts[3], sp_ii, "s4")]
                        for in0, in1, dst, tg in terms:
                            for li in range(2):
                                col = 4 * t + 2 * h + li
                                sg = slice(li * HW, (li + 1) * HW)
                                so = scr.tile([128, HW], DT_MM, tag=tg)
                                nc.vector.scalar_tensor_tensor(
                                    out=so, in0=in0[:, sg], scalar=1.0,
                                    in1=in1[:, sg],
                                    op0=ALU.mult, op1=ALU.mult,
                                    accum_out=dst[:, col:col + 1])

            # ---------------- combine pooled terms -> half spectrum
            # (Pool + ACT: keeps DVE free to finish the product stream)
            p_r = singles.tile([128, NCOL], F32)
            nc.gpsimd.tensor_sub(p_r, sd_rr, sp_ii)
            p_i = singles.tile([128, NCOL], F32)
            nc.gpsimd.tensor_add(p_i, sd_ri, sp_ir)
            # DC bin: bases carry 2/D, k=0 needs 1/D
            nc.scalar.mul(p_r[0:1, 0:4], p_r[0:1, 0:4], 0.5)
            qr = singles.tile([128, 128], DT_MM)
            nc.scalar.copy(qr, p_r[:, 0:128])
            qi = singles.tile([128, 128], DT_MM)
            nc.scalar.mul(qi, p_i[:, 0:128], -1.0)
            qrn = singles.tile([128, 128], DT_MM)
            nc.scalar.mul(qrn, p_r[:, 0:128], -1.0)

            # ---------------- phase B: factored inverse rfft of pooled spectrum
            with tc.tile_pool(name="abp", bufs=2, space="PSUM") as abp, \
                 tc.tile_pool(name="ynp", bufs=1, space="PSUM") as ynp, \
                 tc.tile_pool(name="zp", bufs=2) as zp:
                ypsum = ynp.tile([128, 256], F32)
                # nyquist: pnyq4[i] = P_r[0, 128+i] via 1x4 transpose-matmul
                pny = ynp.tile([4, 1], F32)
                nc.tensor.matmul(pny, lhsT=p_r[0:1, 128:132], rhs=ones1_sb,
                                 start=True, stop=True)
                nyqrow = singles.tile([4, 256], DT_MM)
                nc.vector.tensor_scalar_mul(nyqrow, alt2_sb, pny)

                for chk in range(16):
                    ks = slice(chk * 512, (chk + 1) * 512)
                    a_ps = abp.tile([128, 512], F32, tag="a")
                    b_ps = abp.tile([128, 512], F32, tag="b")
                    nc.tensor.matmul(a_ps, lhsT=qr, rhs=cb_sb[:, ks], start=True, stop=False)
                    nc.tensor.matmul(a_ps, lhsT=qi, rhs=sb_sb[:, ks], start=False, stop=True)
                    nc.tensor.matmul(b_ps, lhsT=qi, rhs=cb_sb[:, ks], start=True, stop=False)
                    nc.tensor.matmul(b_ps, lhsT=qrn, rhs=sb_sb[:, ks], start=False, stop=True)
                    z1 = zp.tile([128, 512], DT_MM, tag="z1")
                    nc.vector.scalar_tensor_tensor(
                        out=z1, in0=a_ps, scalar=1.0, in1=cwf_sb,
                        op0=ALU.mult, op1=ALU.mult)
                    z2 = zp.tile([128, 512], DT_MM, tag="z2")
                    nc.vector.scalar_tensor_tensor(
                        out=z2, in0=b_ps, scalar=1.0, in1=swf_sb,
                        op0=ALU.mult, op1=ALU.mult)
                    z = zp.tile([128, 512], DT_MM, tag="z")
                    # last chunks add on DVE: a trailing Pool add (~1.2us +
                    # drain) would sit on the critical tail
                    if _POOL_ON and chk < 14:
                        nc.gpsimd.tensor_add(z, z1, z2)
                    else:
                        nc.vector.tensor_add(z, z1, z2)
                    # IDFT stage 2: scatter t-sum into (img*32 + n_hi, n_lo)
                    for half in range(2):
                        s = 2 * chk + half
                        nc.tensor.matmul(
                            ypsum,
                            lhsT=selx_sb[:, 31 - s:159 - s],
                            rhs=z[:, half * 256:(half + 1) * 256],
                            start=(s == 0), stop=(s == 31),
                            skip_group_check=True)
                    if chk == 0:
                        # nyquist rank-1 update, scheduled early off the tail
                        nc.tensor.matmul(ypsum, lhsT=nyqs_sb, rhs=nyqrow,
                                         start=False, stop=False,
                                         skip_group_check=True)

                # ------------ epilogue on [128, 256]: signed sqrt + L2 normalize
                t_abs = singles.tile([128, 256], F32)
                # abs with accumulate: row sums come out of the same pass
                rs = singles.tile([128, 1], F32)
                nc.scalar.activation(t_abs, ypsum, ACT.Abs, accum_out=rs)
                sgn = singles.tile([128, 256], F32)
                nc.scalar.activation(sgn, ypsum, ACT.Sign)
                rsb = singles.tile([128, 1], DT_MM)
                nc.vector.tensor_copy(rsb, rs)
                nrm4 = ynp.tile([4, 1], F32)
                nc.tensor.matmul(nrm4, lhsT=bones_sb, rhs=rsb, start=True, stop=True)
                # ||y_ss||^2 = sum(|y| + 1e-8) = sum|y| + D*1e-8
                nrm4s = singles.tile([4, 1], F32)
                nc.scalar.activation(nrm4s, nrm4, ACT.Sqrt, bias=e2)
                inv4 = singles.tile([4, 1], F32)
                nc.vector.reciprocal(inv4, nrm4s)
                inv4b = singles.tile([4, 1], DT_MM)
                nc.vector.tensor_copy(inv4b, inv4)
                binv = ynp.tile([128, 1], F32)
                nc.tensor.matmul(binv, lhsT=nyqs_sb, rhs=inv4b, start=True, stop=True)
                # ss = sqrt(|y| + 1e-8), in place over t_abs
                nc.scalar.activation(t_abs, t_abs, ACT.Sqrt, bias=e1)
                nc.vector.tensor_mul(t_abs, t_abs, sgn)
                nc.vector.tensor_scalar_mul(sgn, t_abs, binv)
                nc.sync.dma_start(
                    out=yd.rearrange("i (r c) -> (i r) c", r=32), in_=sgn)
    return nc


_CACHE = {}


def _enable_axon_tracing():
    """Best-effort NTFF profiling shims for the axon agent image (test-only)."""
    if _CACHE.get("trace_shimmed"):
        return
    import sys
    import types
    try:
        from antenv.axon_hooks import get_axon_ntff_profile_hook  # noqa: F401
    except ImportError:
        try:
            from trn_agent_boot.trn_boot import _ntff_profile_via_ctypes
            hook = _ntff_profile_via_ctypes("/opt/axon/libaxon_pjrt.so")
            m = types.ModuleType("antenv.axon_hooks")
            m.get_axon_ntff_profile_hook = lambda: hook
            m.set_axon_ntff_profile_hook = lambda h: None
            sys.modules["antenv.axon_hooks"] = m
        except Exception as e:  # pragma: no cover
            print("tracing shim unavailable:", e)
    try:
        import concourse.bass_utils as bu
        bu.upload_artifacts = lambda tmpdir: f"local://{tmpdir}"
    except Exception as e:  # pragma: no cover
        print("upload shim failed:", e)
    _CACHE["trace_shimmed"] = True


def _host_consts():
    if "consts" in _CACHE:
        return _CACHE["consts"]
    k1 = np.arange(128, dtype=np.int64)[:, None]
    n = np.arange(D, dtype=np.int64)[None, :]
    ang = 2.0 * np.pi * ((k1 * n) % D) / D
    cbt = (np.cos(ang) * (2.0 / D)).astype(_NPDT)
    sbt = (np.sin(ang) * (2.0 / D)).astype(_NPDT)

    p = np.arange(128, dtype=np.int64)[:, None]
    j = np.arange(512, dtype=np.int64)[None, :]
    ang2 = 2.0 * np.pi * ((p // 4) * (j % 64) % 64) / 64.0
    cwf = np.cos(ang2).astype(np.float32)
    swf = np.sin(ang2).astype(np.float32)

    # selx[p, u] = 1 iff u == (p%4)*32 + 31; slice [31-s : 159-s] selects
    # the scatter matrix for output row-block s
    selx = np.zeros((128, 160), np.float32)
    selx[np.arange(128), (np.arange(128) % 4) * 32 + 31] = 1.0
    # nyqs[i, q] = 1 iff q//32 == i  (image selector / broadcaster)
    nyqs = np.zeros((4, 128), np.float32)
    nyqs[np.arange(128) // 32, np.arange(128)] = 1.0
    alt2 = np.tile(((-1.0) ** np.arange(256)) / D, (4, 1)).astype(np.float32)
    bones = nyqs.T.copy()  # [128, 4] block-ones for the norm reduction
    ones1 = np.ones((1, 1), np.float32)
    _CACHE["consts"] = (cbt, sbt, cwf, swf, selx.astype(_NPDT),
                        nyqs.astype(_NPDT), alt2, bones.astype(_NPDT), ones1)
    return _CACHE["consts"]


def kernel(x, S1, S2):
    x = np.ascontiguousarray(x, dtype=np.float32)
    S1 = np.asarray(S1, dtype=np.float32)
    S2 = np.asarray(S2, dtype=np.float32)

    W1 = np.fft.rfft(S1.astype(np.float64), axis=1)  # [512, 4097]
    W2 = np.fft.rfft(S2.astype(np.float64), axis=1)
    KEXT = NT * 128
    wall = np.zeros((4, CH, KEXT), np.float64)
    wall[0, :, :D // 2] = W1.real[:, :D // 2]
    wall[1, :, :D // 2] = W1.imag[:, :D // 2]
    wall[2, :, :D // 2] = W2.real[:, :D // 2]
    wall[3, :, :D // 2] = W2.imag[:, :D // 2]
    wall[0, :, D // 2] = W1.real[:, D // 2]  # nyquist -> tile 32, col 0
    wall[2, :, D // 2] = W2.real[:, D // 2]
    # tile-major layout: wallt[t, p, m, cc, kk]
    wallt = np.ascontiguousarray(
        wall.reshape(4, 4, 128, NT, 128).transpose(3, 2, 0, 1, 4)
    ).astype(_NPA).reshape(NT, 128, 16, 128)

    cbt, sbt, cwf, swf, selx, nyqs, alt2, bones, ones1 = _host_consts()

    if "nc" not in _CACHE:
        nc = _build()
        nc.finalize()
        _CACHE["nc"] = nc
    nc = _CACHE["nc"]

    common = {
        "wallt": wallt, "cbt": cbt, "sbt": sbt, "cwf": cwf, "swf": swf,
        "selx": selx, "nyqs": nyqs, "alt2": alt2, "bones": bones,
        "ones1": ones1,
    }
    in_maps = []
    for d in range(NCORES):
        xdev = np.ascontiguousarray(
            x[d * BPD:(d + 1) * BPD].transpose(1, 0, 2, 3).reshape(CH, RWS)
            .reshape(4, 128, RWS).transpose(1, 0, 2)
        ).astype(_NPA)
        in_maps.append({"xdev": xdev, **common})

    trace = bool(int(os.environ.get("CBP_TRACE", "0")))
    if trace:
        _enable_axon_tracing()
    res = run_bass_kernel_spmd(nc, in_maps, list(range(NCORES)), trace=trace)
    _CACHE["last_results"] = res
    out = np.concatenate(
        [np.asarray(res.results[d]["ydev"]) for d in range(NCORES)], axis=0
    )
    return out.astype(np.float32)
